# revision 7
# baseline (speedup 1.0000x reference)
"""Trainium2 Bass kernel for a 3-modality grouped BertSelfAttention.

Problem (hardcoded shapes):
  B=4, S=2048, H=768, NH=12 heads of D=64, G=3 modality groups x E=4 heads.
  Group g's input is embeds{g+1}; heads [4g, 4g+4) attend over it.
  out[b, s, h*64:(h+1)*64] = softmax(Q_h K_h^T / 8) V_h  per (b, h).

Sharding (8 cores): core c handles batch b = c//2 and a half of the 12 heads
(6 heads). Halves are chosen so each core needs only 2 of the 3 embeds:
  half 0 -> heads [0,1,2,3, 4,5]   (embeds1 x4, embeds2 x2)
  half 1 -> heads [8,9,10,11, 6,7] (embeds3 x4, embeds2 x2)
Heads are processed in pairs (3 pairs/core); each pair shares one input.

Device-side layout choices:
  - x is fed pre-transposed (xT [H, S], bf16) so projection matmuls contract
    over H on the partition dim with no on-chip transpose.
  - Q,K are produced transposed ([64, S]) packed per pair ([128, S]).
  - Scores are computed transposed (ST[t, s]) so the PV matmul needs no
    transpose; softmax denominators come from a ones-column appended to V
    (V_aug[t, 65], col 64 == 1 after the bias row trick), and the V bias is
    folded in via a ones-row appended to xT (K=1 matmul with the bias row).
  - exp runs on ScalarE straight out of PSUM with the 1/sqrt(D) scale fused.
  - ctx^T [65, S] is PE-transposed back to [s, 65]; col 64 is the denominator,
    normalized via VectorE reciprocal + per-partition tensor_scalar multiply.
"""

import sys

if "/opt/trn_rl_repo" not in sys.path:
    sys.path.insert(0, "/opt/trn_rl_repo")

import math

import ml_dtypes
import numpy as np

import concourse.bass as bass
import concourse.tile as tile
from concourse import bacc, mybir
from concourse.bass_utils import run_bass_kernel_spmd
from concourse.masks import make_identity

B, S, H, NH, D = 4, 2048, 768, 12, 64
SCALE = 1.0 / math.sqrt(D)
HC = H // 128          # 6 contraction chunks of 128
NPAIR = 3              # head pairs per core
SC = 512               # s-chunk for projections / attention streaming
NSC = S // SC          # 4
NTT = S // 128         # 16 t-tiles
BF16 = mybir.dt.bfloat16
F32 = mybir.dt.float32

_CACHE = {}


def _build_nc():
    nc = bacc.Bacc("TRN2", target_bir_lowering=False, debug=False, num_devices=8)

    xa = nc.dram_tensor("xa", [HC, 128, S], BF16, kind="ExternalInput")
    xb = nc.dram_tensor("xb", [HC, 128, S], BF16, kind="ExternalInput")
    wq = nc.dram_tensor("wq", [NPAIR, HC, 128, 128], BF16, kind="ExternalInput")
    wk = nc.dram_tensor("wk", [NPAIR, HC, 128, 128], BF16, kind="ExternalInput")
    wv = nc.dram_tensor("wv", [NPAIR, HC, 128, 130], BF16, kind="ExternalInput")
    bvr = nc.dram_tensor("bvr", [1, NPAIR, 130], BF16, kind="ExternalInput")
    bq = nc.dram_tensor("bq", [NPAIR, 128], F32, kind="ExternalInput")
    bk = nc.dram_tensor("bk", [NPAIR, 128], F32, kind="ExternalInput")
    out = nc.dram_tensor("out", [S, NPAIR * 128], F32, kind="ExternalOutput")

    with tile.TileContext(nc) as tc:
        with (
            tc.tile_pool(name="consts", bufs=1) as consts,
            tc.tile_pool(name="xpool", bufs=1) as xpool,
            tc.tile_pool(name="qkpool", bufs=1) as qkpool,
            tc.tile_pool(name="vpool", bufs=1) as vpool,
            tc.tile_pool(name="epool", bufs=2) as epool,
            tc.tile_pool(name="cpool", bufs=4) as cpool,
            tc.tile_pool(name="opool", bufs=2) as opool,
            tc.tile_pool(name="rpool", bufs=8) as rpool,
            tc.tile_pool(name="st_psum", bufs=2, space="PSUM") as st_psum,
            tc.tile_pool(name="ctx_psum", bufs=2, space="PSUM") as ctx_psum,
            tc.tile_pool(name="tp_psum", bufs=2, space="PSUM") as tp_psum,
        ):
            # ---- constants / weights ----
            ident = consts.tile([128, 128], F32)
            make_identity(nc, ident)
            ones_row = consts.tile([1, 128], BF16)
            nc.vector.memset(ones_row, 1.0)

            wq_sb = consts.tile([128, NPAIR, HC, 128], BF16)
            nc.sync.dma_start(out=wq_sb, in_=wq.rearrange("a c p m -> p a c m"))
            wk_sb = consts.tile([128, NPAIR, HC, 128], BF16)
            nc.sync.dma_start(out=wk_sb, in_=wk.rearrange("a c p m -> p a c m"))
            wv_sb = consts.tile([128, NPAIR, HC, 130], BF16)
            nc.sync.dma_start(out=wv_sb, in_=wv.rearrange("a c p m -> p a c m"))
            bvr_sb = consts.tile([1, NPAIR, 130], BF16)
            nc.sync.dma_start(out=bvr_sb, in_=bvr.ap())
            bq_sb = consts.tile([128, NPAIR], F32)
            nc.sync.dma_start(out=bq_sb, in_=bq.rearrange("a p -> p a"))
            bk_sb = consts.tile([128, NPAIR], F32)
            nc.sync.dma_start(out=bk_sb, in_=bk.rearrange("a p -> p a"))

            x_sb = []  # [128, HC, S] bf16 per input
            for name, dram in (("xa", xa), ("xb", xb)):
                t = xpool.tile([128, HC, S], BF16, tag=f"x_{name}")
                nc.sync.dma_start(out=t, in_=dram.rearrange("c p s -> p c s"))
                x_sb.append(t)

            # ---- phase B: projections ----
            qt_sb, kt_sb, v_sb = [], [], []
            for p in range(NPAIR):
                xs = x_sb[0] if p < 2 else x_sb[1]
                qt = qkpool.tile([128, S], BF16, tag=f"qt{p}")
                kt = qkpool.tile([128, S], BF16, tag=f"kt{p}")
                for sc in range(NSC):
                    ssl = bass.ts(sc, SC)
                    pq = st_psum.tile([128, SC], F32, tag="st")
                    for hc in range(HC):
                        nc.tensor.matmul(
                            pq,
                            wq_sb[:, p, hc, :],
                            xs[:, hc, ssl],
                            start=(hc == 0),
                            stop=(hc == HC - 1),
                        )
                    nc.vector.tensor_scalar(
                        out=qt[:, ssl], in0=pq, scalar1=bq_sb[:, p : p + 1],
                        scalar2=None, op0=mybir.AluOpType.add,
                    )
                    pk = st_psum.tile([128, SC], F32, tag="st")
                    for hc in range(HC):
                        nc.tensor.matmul(
                            pk,
                            wk_sb[:, p, hc, :],
                            xs[:, hc, ssl],
                            start=(hc == 0),
                            stop=(hc == HC - 1),
                        )
                    nc.vector.tensor_scalar(
                        out=kt[:, ssl], in0=pk, scalar1=bk_sb[:, p : p + 1],
                        scalar2=None, op0=mybir.AluOpType.add,
                    )
                qt_sb.append(qt)
                kt_sb.append(kt)

                vt = vpool.tile([128, NTT, 130], BF16, tag=f"v{p}")
                for tt in range(NTT):
                    tsl = bass.ts(tt, 128)
                    pv = st_psum.tile([128, 130], F32, tag="st")
                    for hc in range(HC):
                        nc.tensor.matmul(
                            pv,
                            xs[:, hc, tsl],
                            wv_sb[:, p, hc, :],
                            start=(hc == 0),
                            stop=False,
                        )
                    nc.tensor.matmul(
                        pv, ones_row, bvr_sb[:, p, :], start=False, stop=True,
                    )
                    nc.vector.tensor_copy(vt[:, tt, :], pv)
                v_sb.append(vt)

            # ---- phase C: attention ----
            for sc in range(NSC):
                ssl = bass.ts(sc, SC)
                outs = [opool.tile([128, NPAIR * 128], F32, tag=f"o{st}", name=f"outsb{st}")
                        for st in range(SC // 128)]
                for p in range(NPAIR):
                    e_t = [epool.tile([128, NTT, SC], BF16, tag=f"e{e}", name=f"et{e}")
                           for e in range(2)]
                    for tp in range(NTT // 2):
                        for e in range(2):
                            esl = slice(e * 64, (e + 1) * 64)
                            pst = st_psum.tile([128, 2 * SC], F32, tag="st")
                            for j in range(2):
                                tt = 2 * tp + j
                                nc.tensor.matmul(
                                    pst[:, bass.ts(j, SC)],
                                    kt_sb[p][esl, bass.ts(tt, 128)],
                                    qt_sb[p][esl, ssl],
                                    start=True,
                                    stop=True,
                                    tile_position=(e * 64, 0),
                                )
                            nc.scalar.activation(
                                out=e_t[e][:, 2 * tp : 2 * tp + 2, :],
                                in_=pst.rearrange("p (a b) -> p a b", a=2),
                                func=mybir.ActivationFunctionType.Exp,
                                scale=SCALE,
                            )
                    for e in range(2):
                        pctx = ctx_psum.tile([65, SC], F32, tag="ctx")
                        for tt in range(NTT):
                            nc.tensor.matmul(
                                pctx,
                                v_sb[p][:, tt, bass.ts(e, 65)],
                                e_t[e][:, tt, :],
                                start=(tt == 0),
                                stop=(tt == NTT - 1),
                            )
                        ctxT = cpool.tile([65, SC], F32, tag="ctxT")
                        nc.vector.tensor_copy(ctxT, pctx)
                        for st in range(SC // 128):
                            ptp = tp_psum.tile([128, 65], F32, tag="tp")
                            nc.tensor.transpose(
                                ptp, ctxT[:, bass.ts(st, 128)], ident[0:65, 0:65],
                            )
                            rec = rpool.tile([128, 1], F32, tag="rec")
                            nc.vector.reciprocal(rec, ptp[:, 64:65])
                            nc.vector.tensor_scalar(
                                out=outs[st][:, bass.ds((2 * p + e) * 64, 64)],
                                in0=ptp[:, 0:64],
                                scalar1=rec,
                                scalar2=None,
                                op0=mybir.AluOpType.mult,
                            )
                for st in range(SC // 128):
                    nc.sync.dma_start(
                        out=out[bass.ds(sc * SC + st * 128, 128), :],
                        in_=outs[st],
                    )

    nc.compile()
    return nc


_HALF_HEADS = {0: [0, 1, 2, 3, 4, 5], 1: [8, 9, 10, 11, 6, 7]}


def _prep_core_inputs(c, embeds, Wq, bq, Wk, bk, Wv, bv):
    b, half = divmod(c, 2)
    order = _HALF_HEADS[half]
    ga = 0 if half == 0 else 2
    bf = ml_dtypes.bfloat16

    xa = np.ascontiguousarray(embeds[ga][b].T).astype(bf).reshape(HC, 128, S)
    xb = np.ascontiguousarray(embeds[1][b].T).astype(bf).reshape(HC, 128, S)

    wq_p = np.empty((NPAIR, H, 128), np.float32)
    wk_p = np.empty((NPAIR, H, 128), np.float32)
    wv_p = np.zeros((NPAIR, H, 130), np.float32)
    bvr_p = np.zeros((NPAIR, 130), np.float32)
    bq_p = np.empty((NPAIR, 128), np.float32)
    bk_p = np.empty((NPAIR, 128), np.float32)
    for p in range(NPAIR):
        h1, h2 = order[2 * p], order[2 * p + 1]
        wq_p[p, :, 0:64] = Wq[h1]
        wq_p[p, :, 64:128] = Wq[h2]
        wk_p[p, :, 0:64] = Wk[h1]
        wk_p[p, :, 64:128] = Wk[h2]
        wv_p[p, :, 0:64] = Wv[h1]
        wv_p[p, :, 65:129] = Wv[h2]
        bq_p[p, 0:64] = bq[h1]
        bq_p[p, 64:128] = bq[h2]
        bk_p[p, 0:64] = bk[h1]
        bk_p[p, 64:128] = bk[h2]
        bvr_p[p, 0:64] = bv[h1]
        bvr_p[p, 64] = 1.0
        bvr_p[p, 65:129] = bv[h2]
        bvr_p[p, 129] = 1.0

    return {
        "xa": xa,
        "xb": xb,
        "wq": wq_p.reshape(NPAIR, HC, 128, 128).astype(bf),
        "wk": wk_p.reshape(NPAIR, HC, 128, 128).astype(bf),
        "wv": wv_p.reshape(NPAIR, HC, 128, 130).astype(bf),
        "bvr": bvr_p.astype(bf).reshape(1, NPAIR, 130),
        "bq": bq_p,
        "bk": bk_p,
    }


def kernel(embeds1, embeds2, embeds3, Wq, bq, Wk, bk, Wv, bv, _want_trace=False):
    if "nc" not in _CACHE:
        _CACHE["nc"] = _build_nc()
    nc = _CACHE["nc"]

    embeds = [np.asarray(embeds1), np.asarray(embeds2), np.asarray(embeds3)]
    Wq, bq = np.asarray(Wq), np.asarray(bq)
    Wk, bk = np.asarray(Wk), np.asarray(bk)
    Wv, bv = np.asarray(Wv), np.asarray(bv)

    in_maps = [
        _prep_core_inputs(c, embeds, Wq, bq, Wk, bk, Wv, bv) for c in range(8)
    ]
    res = run_bass_kernel_spmd(
        nc, in_maps, core_ids=list(range(8)), trace=_want_trace,
    )
    _CACHE["last_results"] = res

    full = np.empty((B, S, NH * D), np.float32)
    for c in range(8):
        b, half = divmod(c, 2)
        order = _HALF_HEADS[half]
        o = res.results[c]["out"]
        for j, h in enumerate(order):
            full[b, :, h * 64 : (h + 1) * 64] = o[:, j * 64 : (j + 1) * 64]
    return full
